# revision 16
# baseline (speedup 1.0000x reference)
import os
import sys

import numpy as np

for _p in ("/opt/trn_rl_repo",):
    if _p not in sys.path and os.path.isdir(_p):
        sys.path.append(_p)

N = 1500
A = 64
STD = 0.3
PERSON_IDX = 2
INV2S2 = 1.0 / (2.0 * STD * STD)
SCALE = 2.0 * INV2S2

NCORES = 8
OPC = 188            # objects per core (8*188 = 1504 >= 1500)
NO = OPC * NCORES
KMAX = 24            # person slots per launch
NTILE = 12           # 2 persons per matmul tile
TPB = 4              # tiles per band (row-slots at partitions 0/32/64/96)
NBAND = 3
KROWS = 31           # 10 A-hi + ONE + ln + 10 A-lo + 9 B-lo-compensation
BCOLS = 2 * A        # 128 person-side weight columns (2 persons x 64 actions)
BANDC = BCOLS + OPC  # 316 columns per (tile-slot) band chunk

TCLAMP = 16.0        # |t| clamp; clamped pairs give exp(-inv2s2*(16-1)^2) ~ 0
LNFLOOR = -20000.0   # floor for ln(obj)/SCALE (fp16-safe, exp -> 0)


def _hilo(a):
    hi = a.astype(np.float16)
    lo = (a - hi.astype(np.float32)).astype(np.float16)
    return hi, lo


def _obj_arrays(bbox, scores):
    best = scores.max(axis=1)
    idx = scores.argmax(axis=1)
    person = idx == PERSON_IDX
    obj = np.where(person, 0.0, best).astype(np.float32)

    w = bbox[:, 2] - bbox[:, 0]
    h = bbox[:, 3] - bbox[:, 1]
    cx = bbox[:, 0] + 0.5 * w
    cy = bbox[:, 1] + 0.5 * h

    cx_p = np.zeros(NO, np.float32); cx_p[:N] = cx
    cy_p = np.zeros(NO, np.float32); cy_p[:N] = cy
    lw_p = np.zeros(NO, np.float32); lw_p[:N] = np.log(w)
    lh_p = np.zeros(NO, np.float32); lh_p[:N] = np.log(h)
    lno_p = np.full(NO, LNFLOOR, np.float32)
    pos = obj > 0
    lno_p[:N] = np.where(
        pos, np.maximum(np.log(np.maximum(obj, 1e-38)) / SCALE, LNFLOOR), LNFLOOR
    )
    return person, best, w, h, cx, cy, cx_p, cy_p, lw_p, lh_p, lno_p


def _host_prep(hidx, best, w, h, cx, cy, obj_arr, target_mean):
    """Build per-core blobs [NBAND, 128, BANDC] f16 for one batch of <=KMAX
    persons.  Matmul tile t (= 4*band + slot) covers persons (2t, 2t+1):
      S[(pr,a), o] = sum_c mu_c*enc_c - 0.5*e2 - 0.5*m2 + ln(obj)/SCALE
    device computes exp(SCALE*S); host multiplies by humaness*logits."""
    cx_p, cy_p, lw_p, lh_p, lno_p = obj_arr
    k = len(hidx)

    invw = np.ones(KMAX, np.float32); invw[:k] = 1.0 / w[hidx]
    invh = np.ones(KMAX, np.float32); invh[:k] = 1.0 / h[hidx]
    cxh = np.zeros(KMAX, np.float32); cxh[:k] = cx[hidx] / w[hidx]
    cyh = np.zeros(KMAX, np.float32); cyh[:k] = cy[hidx] / h[hidx]
    lwh = np.zeros(KMAX, np.float32); lwh[:k] = np.log(w[hidx])
    lhh = np.zeros(KMAX, np.float32); lhh[:k] = np.log(h[hidx])
    mu = np.zeros((KMAX, A, 4), np.float32); mu[:k] = target_mean[hidx]
    m2 = (mu * mu).sum(axis=-1)                      # [KMAX, A]

    # encodings for all person-slots x padded objects [KMAX, NO]
    tx = np.clip(cx_p[None, :] * invw[:, None] - cxh[:, None], -TCLAMP, TCLAMP)
    ty = np.clip(cy_p[None, :] * invh[:, None] - cyh[:, None], -TCLAMP, TCLAMP)
    tw = np.clip(lw_p[None, :] - lwh[:, None], -TCLAMP, TCLAMP)
    th = np.clip(lh_p[None, :] - lhh[:, None], -TCLAMP, TCLAMP)
    e2 = tx * tx + ty * ty + tw * tw + th * th

    # A-side (object/streaming) rows [NTILE, KROWS, NO]
    enc = np.stack([tx, ty, tw, th, e2], axis=1)     # [KMAX, 5, NO]
    enc = enc.reshape(NTILE, 10, NO)                 # person-pair tiles
    ehi, elo = _hilo(enc)
    Af = np.zeros((NTILE, KROWS, NO), np.float16)
    Af[:, 0:10] = ehi
    Af[:, 10] = np.float16(1.0)
    Af[:, 11] = lno_p.astype(np.float16)[None, :]
    Af[:, 12:22] = elo
    Af[:, 22:26] = ehi[:, 0:4]                       # tx..th person 0 (hi)
    Af[:, 26:30] = ehi[:, 5:9]                       # tx..th person 1 (hi)
    Af[:, 30] = np.float16(1.0)

    # B-side (person/stationary) weights [NTILE, KROWS, BCOLS]
    muhi, mulo = _hilo(mu)                           # [KMAX, A, 4]
    m2hi, m2lo = _hilo(-0.5 * m2)                    # [KMAX, A]
    Bf = np.zeros((NTILE, KROWS, 2, A), np.float16)
    for pr in range(2):
        mh = muhi[pr::2].reshape(NTILE, A, 4)
        ml = mulo[pr::2].reshape(NTILE, A, 4)
        for c in range(4):
            Bf[:, 5 * pr + c, pr] = mh[:, :, c]
            Bf[:, 22 + 4 * pr + c, pr] = ml[:, :, c]
        Bf[:, 5 * pr + 4, pr] = np.float16(-0.5)
        Bf[:, 10, pr] = m2hi[pr::2].reshape(NTILE, A)
        Bf[:, 30, pr] = m2lo[pr::2].reshape(NTILE, A)
        Bf[:, 11, pr] = np.float16(1.0)
    Bf = Bf.reshape(NTILE, KROWS, BCOLS)
    # rows 12..21 share the hi-row coefficients (A-lo limb x same B)
    Bf[:, 12:22] = Bf[:, 0:10]

    in_maps = []
    for c in range(NCORES):
        blob = np.zeros((NBAND, 128, BANDC), np.float16)
        osl = slice(c * OPC, (c + 1) * OPC)
        for b in range(NBAND):
            for s in range(TPB):
                t = TPB * b + s
                blob[b, 32 * s:32 * s + KROWS, 0:BCOLS] = Bf[t]
                blob[b, 32 * s:32 * s + KROWS, BCOLS:BANDC] = Af[t][:, osl]
        in_maps.append({
            "blob": np.ascontiguousarray(
                blob.transpose(1, 0, 2).reshape(128, NBAND * BANDC)
            ),
        })
    return in_maps


def _gather(results, hidx, best, action_logits, full):
    k = len(hidx)
    lh_ = best[hidx][:, None] * action_logits[hidx]           # [k, A]
    # reassemble per-core tiles into [NBAND, 128, TPB, OPC]
    o0 = np.stack([np.asarray(r["out0"]) for r in results])   # [8,128,4,188]
    o12 = np.stack([np.asarray(r["out12"]) for r in results])  # [8,128,4,2,188]
    big = np.concatenate(
        [o0[:, None], o12.transpose(0, 3, 1, 2, 4)], axis=1
    )                                                          # [8,3,128,4,188]
    g = big.reshape(NCORES, NBAND, 2, A, TPB, OPC)
    g = g.transpose(1, 4, 2, 0, 5, 3)                         # b,s,pr,c,o,a
    g = g.reshape(KMAX, NO, A)[:k, :N, :].astype(np.float32)
    full[hidx] = g * lh_[:, None, :]


_NC_CACHE = {}


def _build_nc():
    if "nc" in _NC_CACHE:
        return _NC_CACHE["nc"]
    import concourse.bacc as bacc
    import concourse.mybir as mybir
    from concourse.tile import TileContext

    f32 = mybir.dt.float32
    f16 = mybir.dt.float16
    nc = bacc.Bacc()
    blob_d = nc.dram_tensor(
        "blob", [128, NBAND * BANDC], f16, kind="ExternalInput"
    )
    out0_d = nc.dram_tensor("out0", [128, TPB, OPC], f16, kind="ExternalOutput")
    # bands 1-2 interleaved: [partition, slot, band-1, objects]
    out12_d = nc.dram_tensor(
        "out12", [128, TPB, 2, OPC], f16, kind="ExternalOutput"
    )

    with TileContext(nc) as tc:
        with (
            tc.tile_pool(name="inp", bufs=3) as ip,
            tc.tile_pool(name="ps", bufs=1, space="PSUM") as pp,
            tc.tile_pool(name="ob", bufs=1) as ob,
        ):
            blobt = ip.tile([128, NBAND * BANDC], f16, tag="blob")
            nc.sync.dma_start(blobt[:], blob_d[:])

            def bsl(b, s, c0, c1):
                return blobt[32 * s:32 * s + KROWS,
                             b * BANDC + c0:b * BANDC + c1]

            # band 0: own 4 banks + early ACT + early out-DMA
            ps0 = pp.tile([128, TPB, 512], f32, tag="ps0")
            for s in range(TPB):
                nc.tensor.matmul(
                    ps0[:, s, 0:OPC],
                    bsl(0, s, 0, BCOLS),
                    bsl(0, s, BCOLS, BANDC),
                    start=True, stop=True,
                    tile_position=(32 * s, 0),
                )
            ot0 = ob.tile([128, TPB, OPC], f16, tag="ot0")
            nc.scalar.activation(
                ot0[:], ps0[:, :, 0:OPC],
                mybir.ActivationFunctionType.Exp, scale=float(SCALE),
            )
            nc.sync.dma_start(out0_d[:], ot0[:])

            # bands 1-2 share 4 banks: tile (s, g) at psum col (2s+g)*256 f32
            # so the 4 concurrent row-slot matmuls always hit distinct banks
            ps2 = pp.tile([128, TPB, 2, 256], f32, tag="ps2")
            for b in (1, 2):
                g = b - 1
                for s in range(TPB):
                    nc.tensor.matmul(
                        ps2[:, s, g, 0:OPC],
                        bsl(b, s, 0, BCOLS),
                        bsl(b, s, BCOLS, BANDC),
                        start=True, stop=True,
                        tile_position=(32 * s, 0),
                    )
            ot2 = ob.tile([128, TPB, 2, OPC], f16, tag="ot2")
            nc.scalar.activation(
                ot2[:], ps2[:, :, :, 0:OPC],
                mybir.ActivationFunctionType.Exp, scale=float(SCALE),
            )
            nc.sync.dma_start(out12_d[:, 0:2], ot2[:, 0:2])
            nc.scalar.dma_start(out12_d[:, 2:4], ot2[:, 2:4])
    nc.finalize()
    _NC_CACHE["nc"] = nc
    return nc


def _run_sim(in_maps):
    results = []
    for m in in_maps:
        blob = m["blob"].reshape(128, NBAND, BANDC).transpose(1, 0, 2)
        out = np.zeros((NBAND, 128, TPB, OPC), np.float16)
        for b in range(NBAND):
            for s in range(TPB):
                Bm = blob[b, 32 * s:32 * s + KROWS, 0:BCOLS].astype(np.float32)
                Am = blob[b, 32 * s:32 * s + KROWS, BCOLS:BANDC].astype(
                    np.float32
                )
                S = Bm.T @ Am
                out[b, :, s, :] = np.exp(
                    np.minimum(SCALE * S, 80.0)
                ).astype(np.float16)
        results.append({
            "out0": out[0],
            "out12": out[1:].transpose(1, 2, 0, 3),
        })
    return results


def kernel(action_logits, target_mean, bbox, scores):
    action_logits = np.asarray(action_logits, np.float32)
    target_mean = np.asarray(target_mean, np.float32)
    bbox = np.asarray(bbox, np.float32)
    scores = np.asarray(scores, np.float32)

    person, best, w, h, cx, cy, cx_p, cy_p, lw_p, lh_p, lno_p = _obj_arrays(
        bbox, scores
    )
    obj_arr = (cx_p, cy_p, lw_p, lh_p, lno_p)
    hidx_all = np.where(person)[0]

    full = np.zeros((N, N, A), np.float32)
    kernel.last_run = None
    for b0 in range(0, len(hidx_all), KMAX):
        hidx = hidx_all[b0:b0 + KMAX]
        in_maps = _host_prep(hidx, best, w, h, cx, cy, obj_arr, target_mean)
        if os.environ.get("KERNEL_SIM") == "1":
            results = _run_sim(in_maps)
        else:
            from concourse.bass_utils import run_bass_kernel_spmd
            nc = _build_nc()
            kw = {}
            if os.environ.get("KERNEL_TRACE") == "1":
                kw = dict(trace=True, trace_cores=list(range(NCORES)))
            r = run_bass_kernel_spmd(
                nc, in_maps, core_ids=list(range(NCORES)), **kw
            )
            results = r.results
            kernel.last_run = r
        _gather(results, hidx, best, action_logits, full)
    return full
